# revision 1
# baseline (speedup 1.0000x reference)
"""Trainium2 Bass kernel for causal multi-head attention with RoPE.

Problem: B=1, S=4096, D=1024, H=16 heads of HD=64.
  q/k/v = x @ w{q,k,v}.T ; rope(q), rope(k); scores = q k^T/sqrt(HD) + mask;
  out = softmax(scores) @ v ; y = out @ wo.T

Sharding: tensor-parallel over heads. 8 cores x 2 heads each.  Each core
computes its 2 heads' q/k/v projections (column-split weights), full
attention for those heads over all 4096 positions, and a partial output
projection (row-split wo).  The host sums the 8 partial [S, D] outputs.

Device layout notes (all "T" tensors have the head-dim on partitions):
  - scores are computed TRANSPOSED: scoresT[sk, sq] so that the
    probs @ v matmul can contract over sk (the partition dim) without
    transposing probabilities.
  - softmax denominator comes from an extra all-ones column appended to V
    (row 64 of the pv PSUM accumulator).
  - The mask is folded in via host-side classification of 128x128 blocks
    of mask.T: all-zero blocks need nothing, all -inf blocks are skipped
    (their probs are exactly 0), mixed blocks get an exact DVE add of the
    original mask values before exp.  This is numerically exact for ANY
    mask; for a causal mask it devolves to a tiny diagonal band of adds.
  - All matmuls run in float32r (full PE rate for N>=256); storage fp32.
"""

import os
import sys

import numpy as np

sys.path.insert(0, "/opt/trn_rl_repo")

S = 4096
D = 1024
H = 16
HD = 64
NCORES = 8
HPC = H // NCORES  # 2 heads per core
EC = HPC * HD  # 128 head-dim columns per core
NEG_THRESH = -1e8  # blocks entirely <= this are "fully masked"

_PROGRAM_CACHE = {}


def classify_mask(maskT, s):
    """Classify 128x128 blocks of mask.T: 0=zero, 1=neginf, 2=general."""
    nb = s // 128
    cls = np.zeros((nb, nb), np.int8)
    for i in range(nb):
        for j in range(nb):
            blk = maskT[128 * i : 128 * (i + 1), 128 * j : 128 * (j + 1)]
            if np.all(blk == 0.0):
                cls[i, j] = 0
            elif np.all(blk <= NEG_THRESH):
                cls[i, j] = 1
            else:
                cls[i, j] = 2
    return cls


def build_program(s, cls, n_gen, neg_bias):
    """Build the SPMD Bass/Tile program for one core (same for all cores).

    s: sequence length; cls: [s/128, s/128] mask block classes (i=sk, j=sq);
    n_gen: number of "general" mask blocks (packed in maskg input);
    neg_bias: constant subtracted inside exp (softmax shift, exact).
    Returns the compiled Bass object.
    """
    from contextlib import ExitStack

    import concourse.bass as bass
    import concourse.tile as tile
    from concourse import bacc, mybir

    f32 = mybir.dt.float32
    f32r = mybir.dt.float32r
    Exp = mybir.ActivationFunctionType.Exp

    nb = s // 128  # sk chunks
    NJ = s // 512  # sq blocks
    nd = D // 128  # contraction chunks for projections

    nc = bacc.Bacc(
        "TRN2", target_bir_lowering=False, debug=False, num_devices=NCORES
    )

    xT = nc.dram_tensor("xT", [D, s], f32r, kind="ExternalInput").ap()
    cosT = nc.dram_tensor("cosT", [128, s], f32, kind="ExternalInput").ap()
    sinTS = nc.dram_tensor("sinTS", [128, s], f32, kind="ExternalInput").ap()
    wqT = nc.dram_tensor("wqT", [128, D], f32r, kind="ExternalInput").ap()
    wkT = nc.dram_tensor("wkT", [128, D], f32r, kind="ExternalInput").ap()
    wvT = nc.dram_tensor("wvT", [128, D], f32r, kind="ExternalInput").ap()
    woT = nc.dram_tensor("woT", [64, 2048], f32r, kind="ExternalInput").ap()
    P128 = nc.dram_tensor("P128", [128, 128], f32r, kind="ExternalInput").ap()
    I128 = nc.dram_tensor("I128", [128, 128], f32, kind="ExternalInput").ap()
    maskg = nc.dram_tensor(
        "maskg", [128, 128 * max(n_gen, 1)], f32, kind="ExternalInput"
    ).ap()
    ones2 = nc.dram_tensor("ones2", [128, 130], f32r, kind="ExternalInput").ap()
    y = nc.dram_tensor("y", [s, D], f32, kind="ExternalOutput").ap()

    r = lambda ap: ap  # tensors are declared float32r end-to-end

    with tile.TileContext(nc) as tc, ExitStack() as ctx:
        consts = ctx.enter_context(tc.tile_pool(name="consts", bufs=1))
        persist = ctx.enter_context(tc.tile_pool(name="persist", bufs=1))
        xt_pool = ctx.enter_context(tc.tile_pool(name="xt", bufs=4))
        wk_pool = ctx.enter_context(tc.tile_pool(name="work", bufs=6))
        probs_pool = ctx.enter_context(tc.tile_pool(name="probs", bufs=8))
        mask_pool = ctx.enter_context(tc.tile_pool(name="maskb", bufs=4))
        attn_pool = ctx.enter_context(tc.tile_pool(name="attn", bufs=3))
        bc_pool = ctx.enter_context(tc.tile_pool(name="bc", bufs=3))
        psum = ctx.enter_context(tc.tile_pool(name="psum", bufs=4, space="PSUM"))

        # ---- constants ----
        c_cos = consts.tile([128, s], f32)
        nc.sync.dma_start(c_cos[:], cosT[:])
        c_sin = consts.tile([128, s], f32)
        nc.sync.dma_start(c_sin[:], sinTS[:])
        c_wq = consts.tile([128, D], f32r)
        nc.sync.dma_start(c_wq[:], wqT[:])
        c_wk = consts.tile([128, D], f32r)
        nc.sync.dma_start(c_wk[:], wkT[:])
        c_wv = consts.tile([128, D], f32r)
        nc.sync.dma_start(c_wv[:], wvT[:])
        c_wo = consts.tile([64, 2048], f32r)
        nc.sync.dma_start(c_wo[:], woT[:])
        c_P = consts.tile([128, 128], f32r)
        nc.sync.dma_start(c_P[:], P128[:])
        c_I = consts.tile([128, 128], f32)
        nc.sync.dma_start(c_I[:], I128[:])
        c_one2 = consts.tile([128, 130], f32r)  # cols 0:2 ones, 2:130 zeros
        nc.sync.dma_start(c_one2[:], ones2[:])

        # ---- persistent activations ----
        qT2 = persist.tile([128, s], f32r)  # [2*64 head rows, s] rope'd & scaled
        kT2 = persist.tile([128, s], f32r)
        v_sb = persist.tile([128, nb * 130], f32r)  # per sk-chunk: [A 64|1|B 64|1]

        # ones columns of v_aug (cols 64 and 129 of each 130-wide chunk)
        ones_view = v_sb[:].rearrange("p (c w) -> p c w", w=130)[:, :, 64:130:65]
        nc.vector.tensor_copy(
            ones_view, c_one2[:, None, 0:2].broadcast_to([128, nb, 2])
        )

        # ---- phase 1a: q/k/v projections (outputs in T layout) ----
        for sb in range(s // 512):
            psq = psum.tile([128, 512], f32, tag="mm", bufs=4)
            psk = psum.tile([128, 512], f32, tag="mm", bufs=4)
            psv = psum.tile([128, 512], f32, tag="mm", bufs=4)
            for dc in range(nd):
                xt = xt_pool.tile([128, 512], f32r)
                nc.sync.dma_start(
                    xt[:], xT[128 * dc : 128 * (dc + 1), 512 * sb : 512 * (sb + 1)]
                )
                first, last = dc == 0, dc == nd - 1
                nc.tensor.matmul(
                    psq[:], r(c_wq[:, 128 * dc : 128 * (dc + 1)]), r(xt[:]),
                    start=first, stop=last,
                )
                nc.tensor.matmul(
                    psk[:], r(c_wk[:, 128 * dc : 128 * (dc + 1)]), r(xt[:]),
                    start=first, stop=last,
                )
                nc.tensor.matmul(
                    psv[:], r(c_wv[:, 128 * dc : 128 * (dc + 1)]), r(xt[:]),
                    start=first, stop=last,
                )
            ssl = slice(512 * sb, 512 * (sb + 1))
            # rope for q and k:  out = raw*cos + swap(raw)*sin_signed
            for ps, dst in ((psq, qT2), (psk, kT2)):
                raw = wk_pool.tile([128, 512], f32r, tag="rope")
                nc.vector.tensor_copy(raw[:], ps[:])
                psw = psum.tile([128, 512], f32, tag="aux", bufs=2)
                nc.tensor.matmul(psw[:], r(c_P[:]), r(raw[:]), start=True, stop=True)
                t1 = wk_pool.tile([128, 512], f32, tag="rope")
                nc.vector.tensor_mul(t1[:], raw[:], c_cos[:, ssl])
                t2 = wk_pool.tile([128, 512], f32, tag="rope")
                nc.vector.tensor_mul(t2[:], psw[:], c_sin[:, ssl])
                nc.vector.tensor_add(dst[:, ssl], t1[:], t2[:])
            # v: psum -> sbuf, then per-128 transpose into natural layout
            vtt = wk_pool.tile([128, 512], f32, tag="vtt", bufs=2)
            nc.vector.tensor_copy(vtt[:], psv[:])
            for k4 in range(4):
                sc = 4 * sb + k4
                pst = psum.tile([128, 512], f32, tag="aux", bufs=2)
                nc.tensor.transpose(
                    pst[:, 0:128], vtt[:, 128 * k4 : 128 * (k4 + 1)], c_I[:]
                )
                nc.vector.tensor_copy(
                    v_sb[:, 130 * sc : 130 * sc + 64], pst[:, 0:64]
                )
                nc.vector.tensor_copy(
                    v_sb[:, 130 * sc + 65 : 130 * sc + 129], pst[:, 64:128]
                )

        # ---- phase 2: attention + output projection, per 512-wide q block ----
        for J in range(NJ):
            jsl = slice(512 * J, 512 * (J + 1))
            kept = [
                i
                for i in range(nb)
                if any(cls[i, 4 * J + u] != 1 for u in range(4))
            ]
            if not kept:
                continue  # fully masked q-block: y rows stay (pre-zeroed) 0
            pvA = psum.tile([128, 512], f32, tag="pv", bufs=2)
            pvB = psum.tile([128, 512], f32, tag="pv", bufs=2)
            # group kept sk-chunks to limit live probs tiles / PE mode swaps
            for g0 in range(0, len(kept), 4):
                group = kept[g0 : g0 + 4]
                gp = []  # (i, probsA, probsB)
                for i in group:
                    isl = slice(128 * i, 128 * (i + 1))
                    psA = psum.tile([128, 512], f32, tag="mm", bufs=4)
                    psB = psum.tile([128, 512], f32, tag="mm", bufs=4)
                    nc.tensor.matmul(
                        psA[:], r(kT2[0:64, isl]), r(qT2[0:64, jsl]),
                        start=True, stop=True, tile_position=(0, 0),
                    )
                    nc.tensor.matmul(
                        psB[:], r(kT2[64:128, isl]), r(qT2[64:128, jsl]),
                        start=True, stop=True, tile_position=(64, 0),
                    )
                    subs = [int(cls[i, 4 * J + u]) for u in range(4)]
                    for u, cu in enumerate(subs):
                        if cu == 2:
                            gi = GEN_INDEX[(i, 4 * J + u)]
                            mb = mask_pool.tile([128, 128], f32)
                            nc.sync.dma_start(
                                mb[:], maskg[:, 128 * gi : 128 * (gi + 1)]
                            )
                            usl = slice(128 * u, 128 * (u + 1))
                            nc.vector.tensor_add(psA[:, usl], psA[:, usl], mb[:])
                            nc.vector.tensor_add(psB[:, usl], psB[:, usl], mb[:])
                    # exp from the first non-masked sub onward
                    fk = min(u for u in range(4) if subs[u] != 1)
                    esl = slice(128 * fk, 512)
                    pA = probs_pool.tile([128, 512], f32r)
                    pB = probs_pool.tile([128, 512], f32r)
                    nc.scalar.activation(pA[:, esl], psA[:, esl], Exp, bias=-neg_bias)
                    nc.scalar.activation(pB[:, esl], psB[:, esl], Exp, bias=-neg_bias)
                    for u, cu in enumerate(subs):
                        if cu == 1:
                            usl = slice(128 * u, 128 * (u + 1))
                            nc.vector.tensor_copy(pA[:, usl], c_one2[:, 2:130])
                            nc.vector.tensor_copy(pB[:, usl], c_one2[:, 2:130])
                    gp.append((i, pA, pB))
                for i, pA, pB in gp:
                    first, last = i == kept[0], i == kept[-1]
                    csl_a = slice(130 * i, 130 * i + 65)
                    csl_b = slice(130 * i + 65, 130 * i + 130)
                    nc.tensor.matmul(
                        pvA[0:65, :], r(v_sb[:, csl_a]), r(pA[:]),
                        start=first, stop=last,
                    )
                    nc.tensor.matmul(
                        pvB[0:65, :], r(v_sb[:, csl_b]), r(pB[:]),
                        start=first, stop=last,
                    )
            # normalize:  attn = pv[0:64] * (1/denom row 64)
            ats = []
            for pv in (pvA, pvB):
                den = wk_pool.tile([128, 512], f32, tag="den", bufs=2)
                nc.vector.tensor_copy(den[64:65, :], pv[64:65, :])
                # custom DVE / gpsimd ops only work at partition base 0 on HW:
                # DMA-shift the row down, then recip + broadcast at base 0.
                d0 = wk_pool.tile([1, 512], f32, tag="d0", bufs=4)
                nc.sync.dma_start(d0[:], den[64:65, :])
                r0 = wk_pool.tile([1, 512], f32, tag="d0", bufs=4)
                s0 = wk_pool.tile([1, 512], f32, tag="d0", bufs=4)
                nc.vector.reciprocal_approx_accurate(r0[:], d0[:], s0[:])
                bc = bc_pool.tile([64, 512], f32)
                nc.gpsimd.partition_broadcast(bc[:], r0[:])
                at = attn_pool.tile([64, 512], f32r)
                nc.vector.tensor_mul(at[:], pv[0:64, :], bc[:])
                ats.append(at)
            atA, atB = ats
            # output projection: y[512J:512J+512, :] partial
            for scn in range(4):
                ssl2 = slice(128 * scn, 128 * (scn + 1))
                rows = slice(512 * J + 128 * scn, 512 * J + 128 * (scn + 1))
                for do2 in range(2):
                    dsl = slice(512 * do2, 512 * (do2 + 1))
                    pso = psum.tile([128, 512], f32, tag="aux", bufs=2)
                    nc.tensor.matmul(
                        pso[:], r(atA[:, ssl2]), r(c_wo[:, dsl]),
                        start=True, stop=False,
                    )
                    nc.tensor.matmul(
                        pso[:], r(atB[:, ssl2]),
                        r(c_wo[:, 1024 + 512 * do2 : 1024 + 512 * (do2 + 1)]),
                        start=False, stop=True,
                    )
                    yo = wk_pool.tile([128, 512], f32, tag="yo", bufs=2)
                    nc.vector.tensor_copy(yo[:], pso[:])
                    nc.sync.dma_start(y[rows, dsl], yo[:])

    nc.compile()
    return nc


GEN_INDEX = {}


def host_prep(x, freqs_cos, freqs_sin, mask, wq, wk, wv, wo):
    """Build per-core input maps + mask classification.  Returns
    (in_maps, cls, n_gen, neg_bias)."""
    global GEN_INDEX
    s = x.shape[1]
    x2 = np.ascontiguousarray(x.reshape(s, D))
    xT = np.ascontiguousarray(x2.T).astype(np.float32)

    # rope tables in T layout (same for q and k; q scale folded into wq)
    p = np.arange(128)
    j = (p % HD) // 2  # freq index per partition row
    cosT = np.ascontiguousarray(freqs_cos.T[j, :]).astype(np.float32)  # [128, s]
    sinT = freqs_sin.T[j, :]
    sign = np.where(p % 2 == 0, -1.0, 1.0).astype(np.float32)
    sinTS = np.ascontiguousarray(sinT * sign[:, None]).astype(np.float32)

    # swap-adjacent permutation and identity
    P = np.zeros((128, 128), np.float32)
    P[np.arange(128) ^ 1, np.arange(128)] = 1.0
    I = np.eye(128, dtype=np.float32)

    # mask classification + general block packing
    maskT = np.ascontiguousarray(mask.T).astype(np.float32)
    cls = classify_mask(maskT, s)
    GEN_INDEX = {}
    gen_blocks = []
    nbk = s // 128
    for i in range(nbk):
        for jj in range(nbk):
            if cls[i, jj] == 2:
                GEN_INDEX[(i, jj)] = len(gen_blocks)
                gen_blocks.append(
                    maskT[128 * i : 128 * (i + 1), 128 * jj : 128 * (jj + 1)]
                )
    n_gen = len(gen_blocks)
    if n_gen:
        maskg = np.ascontiguousarray(np.concatenate(gen_blocks, axis=1))
    else:
        maskg = np.zeros((128, 128), np.float32)

    # softmax shift: exact true max of q k^T / 8 over all heads (BLAS, ~1s)
    qf = x2 @ wq.T
    kf = x2 @ wk.T
    # rope preserves pair norms; compute true scores max per head cheaply
    qh = _rope_np(qf, freqs_cos, freqs_sin)
    kh = _rope_np(kf, freqs_cos, freqs_sin)
    m = 0.0
    for h in range(H):
        qs = qh[:, HD * h : HD * (h + 1)]
        ks = kh[:, HD * h : HD * (h + 1)]
        m = max(m, float(np.abs(qs @ ks.T).max()) / 8.0)
    neg_bias = max(0.0, m - 60.0)

    def pack_w(w_slice):  # [EC, D] -> [128, D] chunked-transpose layout
        t = np.ascontiguousarray(w_slice.T)  # [D, EC=128]
        return np.ascontiguousarray(
            t.reshape(D // 128, 128, 128).transpose(1, 0, 2).reshape(128, D)
        ).astype(np.float32)

    ones130 = np.zeros((128, 130), np.float32)
    ones130[:, 0:2] = 1.0
    in_maps = []
    for c in range(NCORES):
        R = slice(EC * c, EC * (c + 1))
        woc = wo[:, R]  # [D, 128]
        woT_pack = np.concatenate(
            [np.ascontiguousarray(woc[:, 0:64].T), np.ascontiguousarray(woc[:, 64:128].T)],
            axis=1,
        ).astype(np.float32)  # [64, 2048]
        in_maps.append(
            {
                "xT": xT,
                "cosT": cosT,
                "sinTS": sinTS,
                "wqT": pack_w(wq[R] * 0.125),
                "wkT": pack_w(wk[R]),
                "wvT": pack_w(wv[R]),
                "woT": woT_pack,
                "P128": P,
                "I128": I,
                "maskg": maskg,
                "ones2": ones130,
            }
        )
    return in_maps, cls, n_gen, neg_bias


def _rope_np(t, cos, sin):
    s = t.shape[0]
    tr = t.reshape(s, H, HD // 2, 2)
    te, to = tr[..., 0], tr[..., 1]
    c = cos[:, None, :]
    sn = sin[:, None, :]
    oe = te * c - to * sn
    oo = te * sn + to * c
    return np.stack([oe, oo], axis=-1).reshape(s, H * HD)


def kernel(**inputs):
    from concourse.bass_utils import run_bass_kernel_spmd

    x = np.asarray(inputs["x"], np.float32)
    in_maps, cls, n_gen, neg_bias = host_prep(
        x,
        np.asarray(inputs["freqs_cos"], np.float32),
        np.asarray(inputs["freqs_sin"], np.float32),
        np.asarray(inputs["mask"], np.float32),
        np.asarray(inputs["wq"], np.float32),
        np.asarray(inputs["wk"], np.float32),
        np.asarray(inputs["wv"], np.float32),
        np.asarray(inputs["wo"], np.float32),
    )
    key = (x.shape[1], cls.tobytes(), n_gen, float(neg_bias))
    if key not in _PROGRAM_CACHE:
        _PROGRAM_CACHE[key] = build_program(x.shape[1], cls, n_gen, neg_bias)
    nc = _PROGRAM_CACHE[key]
    res = run_bass_kernel_spmd(nc, in_maps, core_ids=list(range(NCORES)))
    y = np.zeros((x.shape[1], D), np.float32)
    for c in range(NCORES):
        y += res.results[c]["y"]
    return y.reshape(x.shape)



# revision 6
# speedup vs baseline: 1.3947x; 1.3947x over previous
"""Trainium2 Bass kernel for causal multi-head attention with RoPE.

Problem: B=1, S=4096, D=1024, H=16 heads of HD=64.
  q/k/v = x @ w{q,k,v}.T ; rope(q), rope(k); scores = q k^T/sqrt(HD) + mask;
  out = softmax(scores) @ v ; y = out @ wo.T

Sharding: tensor-parallel over heads. 8 cores x 2 heads each.  Each core
computes its 2 heads' q/k/v projections (column-split weights), full
attention for those heads over all 4096 positions, and a partial output
projection (row-split wo).  The host sums the 8 partial [S, D] outputs.

v2: all matmul operands in bf16 (fp32r matmuls trip the NeuronCore power
throttle to ~50% duty for most of the run; bf16 also halves DMA and
enables 2x/4x DVE modes).  PSUM accumulation stays fp32, softmax exp is
computed on fp32 scores in PSUM.  The output projection contracts both
heads in one K=128 matmul (attn tiles for head A/B stacked on partitions
0:64 / 64:128 via a SBUF->SBUF DMA shift).  y partials are written bf16
with one consolidated DMA per 512-row block.

Device layout notes (all "T" tensors have the head-dim on partitions):
  - scores are computed TRANSPOSED: scoresT[sk, sq] so that the
    probs @ v matmul can contract over sk (the partition dim) without
    transposing probabilities.
  - softmax denominator comes from an extra all-ones column appended to V
    (row 64 of the pv PSUM accumulator).
  - The mask is folded in via host-side classification of 128x128 blocks
    of mask.T: all-zero blocks need nothing, all -inf blocks are skipped
    (their probs are exactly 0), mixed blocks get an exact DVE add of the
    original mask values before exp.  This is numerically exact for ANY
    mask; for a causal mask it devolves to a tiny diagonal band of adds.
"""

import os
import sys

import numpy as np

sys.path.insert(0, "/opt/trn_rl_repo")

S = 4096
D = 1024
H = 16
HD = 64
NCORES = 8
HPC = H // NCORES  # 2 heads per core
EC = HPC * HD  # 128 head-dim columns per core
NEG_THRESH = -1e8  # blocks entirely <= this are "fully masked"

_PROGRAM_CACHE = {}


def classify_mask(maskT, s):
    """Classify 128x128 blocks of mask.T: 0=zero, 1=neginf, 2=general."""
    nb = s // 128
    cls = np.zeros((nb, nb), np.int8)
    for i in range(nb):
        for j in range(nb):
            blk = maskT[128 * i : 128 * (i + 1), 128 * j : 128 * (j + 1)]
            if np.all(blk == 0.0):
                cls[i, j] = 0
            elif np.all(blk <= NEG_THRESH):
                cls[i, j] = 1
            else:
                cls[i, j] = 2
    return cls


def build_program(s, cls, n_gen, neg_bias):
    """Build the SPMD Bass/Tile program for one core (same for all cores).

    s: sequence length; cls: [s/128, s/128] mask block classes (i=sk, j=sq);
    n_gen: number of "general" mask blocks (packed in maskg input);
    neg_bias: constant subtracted inside exp (softmax shift, exact).
    Returns the compiled Bass object.
    """
    from contextlib import ExitStack

    import concourse.bass as bass
    import concourse.tile as tile
    from concourse import bacc, mybir

    f32 = mybir.dt.float32
    bf16 = mybir.dt.bfloat16
    Exp = mybir.ActivationFunctionType.Exp

    nb = s // 128  # sk chunks
    NJ = s // 512  # sq blocks
    nd = D // 128  # contraction chunks for projections

    nc = bacc.Bacc(
        "TRN2", target_bir_lowering=False, debug=False, num_devices=NCORES
    )

    xT = nc.dram_tensor("xT", [D, s], bf16, kind="ExternalInput").ap()
    cosT = nc.dram_tensor("cosT", [128, s], bf16, kind="ExternalInput").ap()
    sinTS = nc.dram_tensor("sinTS", [128, s], bf16, kind="ExternalInput").ap()
    wqT = nc.dram_tensor("wqT", [128, D], bf16, kind="ExternalInput").ap()
    wkT = nc.dram_tensor("wkT", [128, D], bf16, kind="ExternalInput").ap()
    wvT = nc.dram_tensor("wvT", [128, D], bf16, kind="ExternalInput").ap()
    woT = nc.dram_tensor("woT", [128, D], bf16, kind="ExternalInput").ap()
    P128 = nc.dram_tensor("P128", [128, 128], bf16, kind="ExternalInput").ap()
    I128 = nc.dram_tensor("I128", [128, 128], bf16, kind="ExternalInput").ap()
    maskg = nc.dram_tensor(
        "maskg", [128, 128 * max(n_gen, 1)], f32, kind="ExternalInput"
    ).ap()
    ones2 = nc.dram_tensor("ones2", [128, 130], bf16, kind="ExternalInput").ap()
    y = nc.dram_tensor("y", [s, D], bf16, kind="ExternalOutput").ap()

    with tile.TileContext(nc) as tc, ExitStack() as ctx:
        consts = ctx.enter_context(tc.tile_pool(name="consts", bufs=1))
        persist = ctx.enter_context(tc.tile_pool(name="persist", bufs=1))
        xt_pool = ctx.enter_context(tc.tile_pool(name="xt", bufs=4))
        wk_pool = ctx.enter_context(tc.tile_pool(name="work", bufs=6))
        probs_pool = ctx.enter_context(tc.tile_pool(name="probs", bufs=8))
        mask_pool = ctx.enter_context(tc.tile_pool(name="maskb", bufs=4))
        attn_pool = ctx.enter_context(tc.tile_pool(name="attn", bufs=3))
        bc_pool = ctx.enter_context(tc.tile_pool(name="bc", bufs=3))
        yo_pool = ctx.enter_context(tc.tile_pool(name="yo", bufs=2))
        psum = ctx.enter_context(tc.tile_pool(name="psum", bufs=4, space="PSUM"))

        # ---- constants ----
        c_cos = consts.tile([128, s], bf16)
        nc.sync.dma_start(c_cos[:], cosT[:])
        c_sin = consts.tile([128, s], bf16)
        nc.sync.dma_start(c_sin[:], sinTS[:])
        c_wq = consts.tile([128, D], bf16)
        nc.sync.dma_start(c_wq[:], wqT[:])
        c_wk = consts.tile([128, D], bf16)
        nc.sync.dma_start(c_wk[:], wkT[:])
        c_wv = consts.tile([128, D], bf16)
        nc.sync.dma_start(c_wv[:], wvT[:])
        c_wo = consts.tile([128, D], bf16)
        nc.sync.dma_start(c_wo[:], woT[:])
        c_P = consts.tile([128, 128], bf16)
        nc.sync.dma_start(c_P[:], P128[:])
        c_I = consts.tile([128, 128], bf16)
        nc.sync.dma_start(c_I[:], I128[:])
        c_one2 = consts.tile([128, 130], bf16)  # cols 0:2 ones, 2:130 zeros
        nc.sync.dma_start(c_one2[:], ones2[:])

        # ---- persistent activations ----
        qT2 = persist.tile([128, s], bf16)  # [2*64 head rows, s] rope'd & scaled
        kT2 = persist.tile([128, s], bf16)
        v_sb = persist.tile([128, nb * 130], bf16)  # per sk-chunk: [A 64|1|B 64|1]

        # ones columns of v_aug (cols 64 and 129 of each 130-wide chunk)
        ones_view = v_sb[:].rearrange("p (c w) -> p c w", w=130)[:, :, 64:130:65]
        nc.vector.tensor_copy(
            ones_view, c_one2[:, None, 0:2].broadcast_to([128, nb, 2])
        )

        # ---- phase 1a: q/k/v projections (outputs in T layout) ----
        for sb in range(s // 512):
            psq = psum.tile([128, 512], f32, tag="mm", bufs=4)
            psk = psum.tile([128, 512], f32, tag="mm", bufs=4)
            psv = psum.tile([128, 512], f32, tag="mm", bufs=4)
            for dc in range(nd):
                xt = xt_pool.tile([128, 512], bf16)
                nc.sync.dma_start(
                    xt[:], xT[128 * dc : 128 * (dc + 1), 512 * sb : 512 * (sb + 1)]
                )
                first, last = dc == 0, dc == nd - 1
                nc.tensor.matmul(
                    psq[:], c_wq[:, 128 * dc : 128 * (dc + 1)], xt[:],
                    start=first, stop=last,
                )
                nc.tensor.matmul(
                    psk[:], c_wk[:, 128 * dc : 128 * (dc + 1)], xt[:],
                    start=first, stop=last,
                )
                nc.tensor.matmul(
                    psv[:], c_wv[:, 128 * dc : 128 * (dc + 1)], xt[:],
                    start=first, stop=last,
                )
            ssl = slice(512 * sb, 512 * (sb + 1))
            # rope for q and k:  out = raw*cos + swap(raw)*sin_signed
            for ps, dst in ((psq, qT2), (psk, kT2)):
                raw = wk_pool.tile([128, 512], bf16, tag="rope")
                nc.vector.tensor_copy(raw[:], ps[:])
                psw = psum.tile([128, 512], f32, tag="aux", bufs=2)
                nc.tensor.matmul(psw[:], c_P[:], raw[:], start=True, stop=True)
                t1 = wk_pool.tile([128, 512], bf16, tag="rope")
                nc.vector.tensor_mul(t1[:], raw[:], c_cos[:, ssl])
                t2 = wk_pool.tile([128, 512], bf16, tag="rope")
                nc.vector.tensor_mul(t2[:], psw[:], c_sin[:, ssl])
                nc.vector.tensor_add(dst[:, ssl], t1[:], t2[:])
            # v: psum -> sbuf, then per-128 transpose into natural layout
            vtt = wk_pool.tile([128, 512], bf16, tag="vtt", bufs=2)
            nc.vector.tensor_copy(vtt[:], psv[:])
            for k4 in range(4):
                sc = 4 * sb + k4
                pst = psum.tile([128, 512], bf16, tag="aux", bufs=2)
                nc.tensor.transpose(
                    pst[:, 0:128], vtt[:, 128 * k4 : 128 * (k4 + 1)], c_I[:]
                )
                nc.vector.tensor_copy(
                    v_sb[:, 130 * sc : 130 * sc + 64], pst[:, 0:64]
                )
                nc.vector.tensor_copy(
                    v_sb[:, 130 * sc + 65 : 130 * sc + 129], pst[:, 64:128]
                )

        # ---- phase 2: attention + output projection, per 512-wide q block ----
        for J in range(NJ):
            jsl = slice(512 * J, 512 * (J + 1))
            kept = [
                i
                for i in range(nb)
                if any(cls[i, 4 * J + u] != 1 for u in range(4))
            ]
            if not kept:
                continue  # fully masked q-block: y rows stay (pre-zeroed) 0
            pvA = psum.tile([128, 512], f32, tag="pv", bufs=2)
            pvB = psum.tile([128, 512], f32, tag="pv", bufs=2)
            # group kept sk-chunks to limit live probs tiles / PE mode swaps
            for g0 in range(0, len(kept), 4):
                group = kept[g0 : g0 + 4]
                gp = []  # (i, probsA, probsB)
                for i in group:
                    isl = slice(128 * i, 128 * (i + 1))
                    psA = psum.tile([128, 512], f32, tag="mm", bufs=4)
                    psB = psum.tile([128, 512], f32, tag="mm", bufs=4)
                    nc.tensor.matmul(
                        psA[:], kT2[0:64, isl], qT2[0:64, jsl],
                        start=True, stop=True, tile_position=(0, 0),
                    )
                    nc.tensor.matmul(
                        psB[:], kT2[64:128, isl], qT2[64:128, jsl],
                        start=True, stop=True, tile_position=(64, 0),
                    )
                    subs = [int(cls[i, 4 * J + u]) for u in range(4)]
                    for u, cu in enumerate(subs):
                        if cu == 2:
                            gi = GEN_INDEX[(i, 4 * J + u)]
                            mb = mask_pool.tile([128, 128], f32)
                            nc.sync.dma_start(
                                mb[:], maskg[:, 128 * gi : 128 * (gi + 1)]
                            )
                            usl = slice(128 * u, 128 * (u + 1))
                            nc.vector.tensor_add(psA[:, usl], psA[:, usl], mb[:])
                            nc.vector.tensor_add(psB[:, usl], psB[:, usl], mb[:])
                    # exp from the first non-masked sub onward
                    fk = min(u for u in range(4) if subs[u] != 1)
                    esl = slice(128 * fk, 512)
                    pA = probs_pool.tile([128, 512], bf16)
                    pB = probs_pool.tile([128, 512], bf16)
                    nc.scalar.activation(pA[:, esl], psA[:, esl], Exp, bias=-neg_bias)
                    nc.scalar.activation(pB[:, esl], psB[:, esl], Exp, bias=-neg_bias)
                    for u, cu in enumerate(subs):
                        if cu == 1:
                            usl = slice(128 * u, 128 * (u + 1))
                            nc.vector.tensor_copy(pA[:, usl], c_one2[:, 2:130])
                            nc.vector.tensor_copy(pB[:, usl], c_one2[:, 2:130])
                    gp.append((i, pA, pB))
                for i, pA, pB in gp:
                    first, last = i == kept[0], i == kept[-1]
                    csl_a = slice(130 * i, 130 * i + 65)
                    csl_b = slice(130 * i + 65, 130 * i + 130)
                    nc.tensor.matmul(
                        pvA[0:65, :], v_sb[:, csl_a], pA[:],
                        start=first, stop=last,
                    )
                    nc.tensor.matmul(
                        pvB[0:65, :], v_sb[:, csl_b], pB[:],
                        start=first, stop=last,
                    )
            # normalize:  attn = pv[0:64] * (1/denom row 64); head A lands in
            # at[0:64] directly, head B goes via a SBUF->SBUF DMA partition
            # shift into at[64:128] so the output projection can contract
            # both heads in one K=128 matmul.
            at = attn_pool.tile([128, 512], bf16)
            for hb, pv in ((0, pvA), (1, pvB)):
                den = wk_pool.tile([128, 512], f32, tag="den", bufs=2)
                nc.vector.tensor_copy(den[64:65, :], pv[64:65, :])
                # custom DVE / gpsimd ops only work at partition base 0 on HW:
                # DMA-shift the row down, then recip + broadcast at base 0.
                d0 = wk_pool.tile([1, 512], f32, tag="d0", bufs=4)
                nc.sync.dma_start(d0[:], den[64:65, :])
                r0 = wk_pool.tile([1, 512], f32, tag="d0", bufs=4)
                s0 = wk_pool.tile([1, 512], f32, tag="d0", bufs=4)
                nc.vector.reciprocal_approx_accurate(r0[:], d0[:], s0[:])
                bc = bc_pool.tile([64, 512], f32)
                nc.gpsimd.partition_broadcast(bc[:], r0[:])
                if hb == 0:
                    nc.vector.tensor_mul(at[0:64, :], pv[0:64, :], bc[:])
                else:
                    atB = attn_pool.tile([64, 512], bf16)
                    nc.vector.tensor_mul(atB[:], pv[0:64, :], bc[:])
                    nc.sync.dma_start(at[64:128, :], atB[:])
            # output projection: y[512J:512J+512, :] partial, K=128 over both
            # heads; results collected in one yo tile, one DMA per J.
            yo = yo_pool.tile([128, 4096], bf16)
            yJ = y[512 * J : 512 * (J + 1), :].rearrange(
                "(c p) d -> p c d", p=128
            )
            for scn in range(4):
                ssl2 = slice(128 * scn, 128 * (scn + 1))
                for do2 in range(2):
                    dsl = slice(512 * do2, 512 * (do2 + 1))
                    pso = psum.tile([128, 512], f32, tag="aux", bufs=2)
                    nc.tensor.matmul(
                        pso[:], at[:, ssl2], c_wo[:, dsl],
                        start=True, stop=True,
                    )
                    osl = slice(1024 * scn + 512 * do2, 1024 * scn + 512 * do2 + 512)
                    nc.vector.tensor_copy(yo[:, osl], pso[:])
            nc.sync.dma_start(yJ, yo[:].rearrange("p (c d) -> p c d", d=1024))

    nc.compile()
    return nc


GEN_INDEX = {}


def host_prep(x, freqs_cos, freqs_sin, mask, wq, wk, wv, wo):
    """Build per-core input maps + mask classification.  Returns
    (in_maps, cls, n_gen, neg_bias)."""
    global GEN_INDEX
    import ml_dtypes

    bf16 = ml_dtypes.bfloat16
    s = x.shape[1]
    x2 = np.ascontiguousarray(x.reshape(s, D))
    xT = np.ascontiguousarray(x2.T).astype(bf16)

    # rope tables in T layout (same for q and k; q scale folded into wq)
    p = np.arange(128)
    j = (p % HD) // 2  # freq index per partition row
    cosT = np.ascontiguousarray(freqs_cos.T[j, :]).astype(bf16)  # [128, s]
    sinT = freqs_sin.T[j, :]
    sign = np.where(p % 2 == 0, -1.0, 1.0).astype(np.float32)
    sinTS = np.ascontiguousarray(sinT * sign[:, None]).astype(bf16)

    # swap-adjacent permutation and identity
    P = np.zeros((128, 128), np.float32)
    P[np.arange(128) ^ 1, np.arange(128)] = 1.0
    I = np.eye(128, dtype=np.float32)

    # mask classification + general block packing
    maskT = np.ascontiguousarray(mask.T).astype(np.float32)
    cls = classify_mask(maskT, s)
    GEN_INDEX = {}
    gen_blocks = []
    nbk = s // 128
    for i in range(nbk):
        for jj in range(nbk):
            if cls[i, jj] == 2:
                GEN_INDEX[(i, jj)] = len(gen_blocks)
                gen_blocks.append(
                    maskT[128 * i : 128 * (i + 1), 128 * jj : 128 * (jj + 1)]
                )
    n_gen = len(gen_blocks)
    if n_gen:
        maskg = np.ascontiguousarray(np.concatenate(gen_blocks, axis=1))
    else:
        maskg = np.zeros((128, 128), np.float32)

    # softmax shift: exact true max of q k^T / 8 over all heads (BLAS, ~1s)
    qf = x2 @ wq.T
    kf = x2 @ wk.T
    # rope preserves pair norms; compute true scores max per head cheaply
    qh = _rope_np(qf, freqs_cos, freqs_sin)
    kh = _rope_np(kf, freqs_cos, freqs_sin)
    m = 0.0
    for h in range(H):
        qs = qh[:, HD * h : HD * (h + 1)]
        ks = kh[:, HD * h : HD * (h + 1)]
        m = max(m, float(np.abs(qs @ ks.T).max()) / 8.0)
    neg_bias = max(0.0, m - 60.0)

    def pack_w(w_slice):  # [EC, D] -> [128, D] chunked-transpose layout
        t = np.ascontiguousarray(w_slice.T)  # [D, EC=128]
        return np.ascontiguousarray(
            t.reshape(D // 128, 128, 128).transpose(1, 0, 2).reshape(128, D)
        ).astype(bf16)

    ones130 = np.zeros((128, 130), np.float32)
    ones130[:, 0:2] = 1.0
    in_maps = []
    for c in range(NCORES):
        R = slice(EC * c, EC * (c + 1))
        woc = wo[:, R]  # [D, 128]
        woT_pack = np.ascontiguousarray(woc.T).astype(bf16)  # [128, D]
        in_maps.append(
            {
                "xT": xT,
                "cosT": cosT,
                "sinTS": sinTS,
                "wqT": pack_w(wq[R] * 0.125),
                "wkT": pack_w(wk[R]),
                "wvT": pack_w(wv[R]),
                "woT": woT_pack,
                "P128": P.astype(bf16),
                "I128": I.astype(bf16),
                "maskg": maskg,
                "ones2": ones130.astype(bf16),
            }
        )
    return in_maps, cls, n_gen, neg_bias


def _rope_np(t, cos, sin):
    s = t.shape[0]
    tr = t.reshape(s, H, HD // 2, 2)
    te, to = tr[..., 0], tr[..., 1]
    c = cos[:, None, :]
    sn = sin[:, None, :]
    oe = te * c - to * sn
    oo = te * sn + to * c
    return np.stack([oe, oo], axis=-1).reshape(s, H * HD)


def kernel(**inputs):
    from concourse.bass_utils import run_bass_kernel_spmd

    x = np.asarray(inputs["x"], np.float32)
    in_maps, cls, n_gen, neg_bias = host_prep(
        x,
        np.asarray(inputs["freqs_cos"], np.float32),
        np.asarray(inputs["freqs_sin"], np.float32),
        np.asarray(inputs["mask"], np.float32),
        np.asarray(inputs["wq"], np.float32),
        np.asarray(inputs["wk"], np.float32),
        np.asarray(inputs["wv"], np.float32),
        np.asarray(inputs["wo"], np.float32),
    )
    key = (x.shape[1], cls.tobytes(), n_gen, float(neg_bias))
    if key not in _PROGRAM_CACHE:
        _PROGRAM_CACHE[key] = build_program(x.shape[1], cls, n_gen, neg_bias)
    nc = _PROGRAM_CACHE[key]
    res = run_bass_kernel_spmd(nc, in_maps, core_ids=list(range(NCORES)))
    y = np.zeros((x.shape[1], D), np.float32)
    for c in range(NCORES):
        y += np.asarray(res.results[c]["y"], np.float32)
    return y.reshape(x.shape)


# revision 9
# speedup vs baseline: 1.5903x; 1.1402x over previous
"""Trainium2 Bass kernel for causal multi-head attention with RoPE.

Problem: B=1, S=4096, D=1024, H=16 heads of HD=64.
  q/k/v = x @ w{q,k,v}.T ; rope(q), rope(k); scores = q k^T/sqrt(HD) + mask;
  out = softmax(scores) @ v ; y = out @ wo.T

Sharding: tensor-parallel over heads. 8 cores x 2 heads each.  Each core
computes its 2 heads' q/k/v projections (column-split weights), full
attention for those heads over all 4096 positions, and a partial output
projection (row-split wo).  The host sums the 8 partial [S, D] outputs.

v3 structure (all matmul operands bf16, PSUM accumulation fp32):
  - Projections and attention are INTERLEAVED per 512-row block: thanks to
    causality, the attention for q-block J only needs k/v blocks <= J, so
    attn(J) is emitted right after projection block sb=J.  This hides the
    whole projection phase under the softmax/PE pipeline.
  - Scores for both heads land in one double-width [128, 1024] PSUM tile
    (2 banks) so a single ACT exp instruction covers both heads: 144
    instead of 288 activations (the ACT engine is the softmax bottleneck).
  - The output projection contracts both heads in one K=128 matmul
    (attn A/B stacked on partitions via a SBUF->SBUF DMA shift) and its
    matmuls+casts are emitted one J late, interleaved between score
    chunks, so their PSUM->SBUF casts hide under attention compute.
  - y partials are written bf16, one consolidated DMA per 512-row block.
  - PSUM budget: proj/swap/transpose/outproj rotate in 2 banks, scores
    2x2 banks, pv accumulators 2 banks = 8 banks exactly.
"""

import os
import sys

import numpy as np

sys.path.insert(0, "/opt/trn_rl_repo")

S = 4096
D = 1024
H = 16
HD = 64
NCORES = 8
HPC = H // NCORES  # 2 heads per core
EC = HPC * HD  # 128 head-dim columns per core
NEG_THRESH = -1e8  # blocks entirely <= this are "fully masked"

_PROGRAM_CACHE = {}


def classify_mask(maskT, s):
    """Classify 128x128 blocks of mask.T: 0=zero, 1=neginf, 2=general."""
    nb = s // 128
    cls = np.zeros((nb, nb), np.int8)
    for i in range(nb):
        for j in range(nb):
            blk = maskT[128 * i : 128 * (i + 1), 128 * j : 128 * (j + 1)]
            if np.all(blk == 0.0):
                cls[i, j] = 0
            elif np.all(blk <= NEG_THRESH):
                cls[i, j] = 1
            else:
                cls[i, j] = 2
    return cls


def build_program(s, cls, n_gen, neg_bias):
    """Build the SPMD Bass/Tile program for one core (same for all cores)."""
    from contextlib import ExitStack

    import concourse.bass as bass
    import concourse.tile as tile
    from concourse import bacc, mybir

    f32 = mybir.dt.float32
    bf16 = mybir.dt.bfloat16
    Exp = mybir.ActivationFunctionType.Exp

    nb = s // 128  # sk chunks
    NJ = s // 512  # sq blocks
    nd = D // 128  # contraction chunks for projections

    nc = bacc.Bacc(
        "TRN2", target_bir_lowering=False, debug=False, num_devices=NCORES
    )

    xT = nc.dram_tensor("xT", [D, s], bf16, kind="ExternalInput").ap()
    cosT = nc.dram_tensor("cosT", [128, s], bf16, kind="ExternalInput").ap()
    sinTS = nc.dram_tensor("sinTS", [128, s], bf16, kind="ExternalInput").ap()
    wqT = nc.dram_tensor("wqT", [128, D], bf16, kind="ExternalInput").ap()
    wkT = nc.dram_tensor("wkT", [128, D], bf16, kind="ExternalInput").ap()
    wvT = nc.dram_tensor("wvT", [128, D], bf16, kind="ExternalInput").ap()
    woT = nc.dram_tensor("woT", [128, D], bf16, kind="ExternalInput").ap()
    P128 = nc.dram_tensor("P128", [128, 128], bf16, kind="ExternalInput").ap()
    I128 = nc.dram_tensor("I128", [128, 128], bf16, kind="ExternalInput").ap()
    maskg = nc.dram_tensor(
        "maskg", [128, 128 * max(n_gen, 1)], f32, kind="ExternalInput"
    ).ap()
    ones2 = nc.dram_tensor("ones2", [128, 130], bf16, kind="ExternalInput").ap()
    y = nc.dram_tensor("y", [s, D], bf16, kind="ExternalOutput").ap()

    with tile.TileContext(nc) as tc, ExitStack() as ctx:
        consts = ctx.enter_context(tc.tile_pool(name="consts", bufs=1))
        persist = ctx.enter_context(tc.tile_pool(name="persist", bufs=1))
        xt_pool = ctx.enter_context(tc.tile_pool(name="xt", bufs=2))
        wk_pool = ctx.enter_context(tc.tile_pool(name="work", bufs=6))
        probs_pool = ctx.enter_context(tc.tile_pool(name="probs", bufs=6))
        mask_pool = ctx.enter_context(tc.tile_pool(name="maskb", bufs=4))
        attn_pool = ctx.enter_context(tc.tile_pool(name="attn", bufs=3))
        bc_pool = ctx.enter_context(tc.tile_pool(name="bc", bufs=3))
        yo_pool = ctx.enter_context(tc.tile_pool(name="yo", bufs=2))
        psum = ctx.enter_context(tc.tile_pool(name="psum", bufs=2, space="PSUM"))

        # ---- constants (weights first: the first projection needs them) ----
        c_wq = consts.tile([128, D], bf16)
        nc.sync.dma_start(c_wq[:], wqT[:])
        c_wk = consts.tile([128, D], bf16)
        nc.sync.dma_start(c_wk[:], wkT[:])
        c_wv = consts.tile([128, D], bf16)
        nc.sync.dma_start(c_wv[:], wvT[:])
        c_P = consts.tile([128, 128], bf16)
        nc.sync.dma_start(c_P[:], P128[:])
        c_I = consts.tile([128, 128], bf16)
        nc.sync.dma_start(c_I[:], I128[:])
        c_cos = consts.tile([128, s], bf16)
        nc.sync.dma_start(c_cos[:], cosT[:])
        c_sin = consts.tile([128, s], bf16)
        nc.sync.dma_start(c_sin[:], sinTS[:])
        c_one2 = consts.tile([128, 130], bf16)  # cols 0:2 ones, 2:130 zeros
        nc.sync.dma_start(c_one2[:], ones2[:])
        c_wo = consts.tile([128, D], bf16)
        nc.sync.dma_start(c_wo[:], woT[:])

        # ---- persistent activations ----
        qT2 = persist.tile([128, s], bf16)  # [2*64 head rows, s] rope'd & scaled
        kT2 = persist.tile([128, s], bf16)
        v_sb = persist.tile([128, nb * 130], bf16)  # per sk-chunk: [A 64|1|B 64|1]

        # ones columns of v_aug (cols 64 and 129 of each 130-wide chunk)
        ones_view = v_sb[:].rearrange("p (c w) -> p c w", w=130)[:, :, 64:130:65]
        nc.vector.tensor_copy(
            ones_view, c_one2[:, None, 0:2].broadcast_to([128, nb, 2])
        )

        xT_v = xT.rearrange("(c p) sl -> p c sl", p=128)

        def phase1(sb):
            """Projections + rope + v-transpose for rows [512*sb, 512*sb+512)."""
            ssl = slice(512 * sb, 512 * (sb + 1))
            xts = xt_pool.tile([128, nd * 512], bf16)
            nc.sync.dma_start(
                xts[:].rearrange("p (c sl) -> p c sl", sl=512), xT_v[:, :, ssl]
            )
            # q and k passes, each followed by rope
            for cw, dst in ((c_wq, qT2), (c_wk, kT2)):
                ps = psum.tile([128, 512], f32, tag="proj", bufs=2)
                for dc in range(nd):
                    nc.tensor.matmul(
                        ps[:], cw[:, 128 * dc : 128 * (dc + 1)],
                        xts[:, 512 * dc : 512 * (dc + 1)],
                        start=dc == 0, stop=dc == nd - 1,
                    )
                raw = wk_pool.tile([128, 512], bf16, tag="rope")
                nc.vector.tensor_copy(raw[:], ps[:])
                psw = psum.tile([128, 512], f32, tag="proj", bufs=2)
                nc.tensor.matmul(psw[:], c_P[:], raw[:], start=True, stop=True)
                t1 = wk_pool.tile([128, 512], bf16, tag="rope")
                nc.vector.tensor_mul(t1[:], raw[:], c_cos[:, ssl])
                t2 = wk_pool.tile([128, 512], bf16, tag="rope")
                nc.vector.tensor_mul(t2[:], psw[:], c_sin[:, ssl])
                nc.vector.tensor_add(dst[:, ssl], t1[:], t2[:])
            # v pass: psum -> sbuf, then per-128 transpose into natural layout
            psv = psum.tile([128, 512], f32, tag="proj", bufs=2)
            for dc in range(nd):
                nc.tensor.matmul(
                    psv[:], c_wv[:, 128 * dc : 128 * (dc + 1)],
                    xts[:, 512 * dc : 512 * (dc + 1)],
                    start=dc == 0, stop=dc == nd - 1,
                )
            vtt = wk_pool.tile([128, 512], bf16, tag="vtt", bufs=2)
            nc.vector.tensor_copy(vtt[:], psv[:])
            for k4 in range(4):
                sc = 4 * sb + k4
                pst = psum.tile([128, 1024], bf16, tag="proj", bufs=2)
                nc.tensor.transpose(
                    pst[:, 0:128], vtt[:, 128 * k4 : 128 * (k4 + 1)], c_I[:]
                )
                nc.vector.tensor_copy(
                    v_sb[:, 130 * sc : 130 * sc + 64], pst[:, 0:64]
                )
                nc.vector.tensor_copy(
                    v_sb[:, 130 * sc + 65 : 130 * sc + 129], pst[:, 64:128]
                )

        def attn(J, slots):
            """Attention for q rows [512J, 512J+512); interleaves `slots`
            (previous J's output-projection emitters) between score chunks.
            Returns this J's outproj slots."""
            jsl = slice(512 * J, 512 * (J + 1))
            kept = [
                i
                for i in range(nb)
                if any(cls[i, 4 * J + u] != 1 for u in range(4))
            ]
            if not kept:
                while slots:
                    slots.pop(0)()
                return []
            pvA = psum.tile([128, 512], f32, tag="pv", bufs=2)
            pvB = psum.tile([128, 512], f32, tag="pv", bufs=2)
            pvq = []  # chunks whose pv matmuls are not yet emitted

            def emit_pv():
                i, pAB = pvq.pop(0)
                first, last = i == kept[0], i == kept[-1]
                nc.tensor.matmul(
                    pvA[0:65, :], v_sb[:, 130 * i : 130 * i + 65],
                    pAB[:, 0:512], start=first, stop=last,
                )
                nc.tensor.matmul(
                    pvB[0:65, :], v_sb[:, 130 * i + 65 : 130 * i + 130],
                    pAB[:, 512:1024], start=first, stop=last,
                )

            nslot = max(1, -(-len(slots) // len(kept))) if slots else 0
            for n, i in enumerate(kept):
                isl = slice(128 * i, 128 * (i + 1))
                psAB = psum.tile([128, 1024], f32, tag="sc", bufs=2)
                nc.tensor.matmul(
                    psAB[:, 0:512], kT2[0:64, isl], qT2[0:64, jsl],
                    start=True, stop=True, tile_position=(0, 0),
                )
                nc.tensor.matmul(
                    psAB[:, 512:1024], kT2[64:128, isl], qT2[64:128, jsl],
                    start=True, stop=True, tile_position=(64, 0),
                )
                for _ in range(nslot):
                    if slots:
                        slots.pop(0)()
                if len(pvq) >= 2:
                    emit_pv()
                subs = [int(cls[i, 4 * J + u]) for u in range(4)]
                for u, cu in enumerate(subs):
                    if cu == 2:
                        gi = GEN_INDEX[(i, 4 * J + u)]
                        mb = mask_pool.tile([128, 128], f32)
                        nc.sync.dma_start(
                            mb[:], maskg[:, 128 * gi : 128 * (gi + 1)]
                        )
                        for off in (0, 512):
                            usl = slice(off + 128 * u, off + 128 * (u + 1))
                            nc.vector.tensor_add(
                                psAB[:, usl], psAB[:, usl], mb[:]
                            )
                # one exp for both heads; masked subs get zero-filled after
                fk = min(u for u in range(4) if subs[u] != 1)
                esl = slice(128 * fk, 1024)
                pAB = probs_pool.tile([128, 1024], bf16)
                nc.scalar.activation(pAB[:, esl], psAB[:, esl], Exp, bias=-neg_bias)
                for u, cu in enumerate(subs):
                    if cu == 1:
                        for off in (0, 512):
                            usl = slice(off + 128 * u, off + 128 * (u + 1))
                            nc.vector.tensor_copy(pAB[:, usl], c_one2[:, 2:130])
                pvq.append((i, pAB))
            while pvq:
                emit_pv()
            while slots:
                slots.pop(0)()
            # normalize:  attn = pv[0:64] * (1/denom row 64); head A lands in
            # at[0:64] directly, head B goes via a SBUF->SBUF DMA partition
            # shift into at[64:128] so the output projection can contract
            # both heads in one K=128 matmul.
            at = attn_pool.tile([128, 512], bf16)
            for hb, pv in ((0, pvA), (1, pvB)):
                den = wk_pool.tile([128, 512], f32, tag="den", bufs=2)
                nc.vector.tensor_copy(den[64:65, :], pv[64:65, :])
                d0 = wk_pool.tile([1, 512], f32, tag="d0", bufs=4)
                nc.sync.dma_start(d0[:], den[64:65, :])
                r0 = wk_pool.tile([1, 512], f32, tag="d0", bufs=4)
                s0 = wk_pool.tile([1, 512], f32, tag="d0", bufs=4)
                nc.vector.reciprocal_approx_accurate(r0[:], d0[:], s0[:])
                bc = bc_pool.tile([64, 512], f32)
                nc.gpsimd.partition_broadcast(bc[:], r0[:])
                if hb == 0:
                    nc.vector.tensor_mul(at[0:64, :], pv[0:64, :], bc[:])
                else:
                    atB = attn_pool.tile([64, 512], bf16)
                    nc.vector.tensor_mul(atB[:], pv[0:64, :], bc[:])
                    nc.sync.dma_start(at[64:128, :], atB[:])
            # build outproj slot closures for this J (emitted during J+1)
            yo = yo_pool.tile([128, 4096], bf16)
            yJ = y[512 * J : 512 * (J + 1), :].rearrange("(c p) d -> p c d", p=128)
            nleft = [8]

            def mk(scn, do2):
                def emit():
                    ssl2 = slice(128 * scn, 128 * (scn + 1))
                    dsl = slice(512 * do2, 512 * (do2 + 1))
                    pso = psum.tile([128, 512], f32, tag="proj", bufs=2)
                    nc.tensor.matmul(
                        pso[:], at[:, ssl2], c_wo[:, dsl], start=True, stop=True
                    )
                    osl = slice(
                        1024 * scn + 512 * do2, 1024 * scn + 512 * do2 + 512
                    )
                    nc.vector.tensor_copy(yo[:, osl], pso[:])
                    nleft[0] -= 1
                    if nleft[0] == 0:
                        nc.sync.dma_start(
                            yJ, yo[:].rearrange("p (c d) -> p c d", d=1024)
                        )

                return emit

            return [mk(scn, do2) for scn in range(4) for do2 in range(2)]

        # attn(J) reads k/v blocks up to max(kept); interleaving phase1/attn
        # is only legal when that never exceeds the just-written block sb=J.
        causal_ok = all(
            i <= 4 * J + 3
            for J in range(NJ)
            for i in range(nb)
            if any(cls[i, 4 * J + u] != 1 for u in range(4))
        )
        slots = []
        if causal_ok:
            for sb in range(NJ):
                phase1(sb)
                slots = attn(sb, slots)
        else:
            for sb in range(NJ):
                phase1(sb)
            for J in range(NJ):
                slots = attn(J, slots)
        while slots:
            slots.pop(0)()

    nc.compile()
    return nc


GEN_INDEX = {}


def host_prep(x, freqs_cos, freqs_sin, mask, wq, wk, wv, wo):
    """Build per-core input maps + mask classification.  Returns
    (in_maps, cls, n_gen, neg_bias)."""
    global GEN_INDEX
    import ml_dtypes

    bf16 = ml_dtypes.bfloat16
    s = x.shape[1]
    x2 = np.ascontiguousarray(x.reshape(s, D))
    xT = np.ascontiguousarray(x2.T).astype(bf16)

    # rope tables in T layout (same for q and k; q scale folded into wq)
    p = np.arange(128)
    j = (p % HD) // 2  # freq index per partition row
    cosT = np.ascontiguousarray(freqs_cos.T[j, :]).astype(bf16)  # [128, s]
    sinT = freqs_sin.T[j, :]
    sign = np.where(p % 2 == 0, -1.0, 1.0).astype(np.float32)
    sinTS = np.ascontiguousarray(sinT * sign[:, None]).astype(bf16)

    # swap-adjacent permutation and identity
    P = np.zeros((128, 128), np.float32)
    P[np.arange(128) ^ 1, np.arange(128)] = 1.0
    I = np.eye(128, dtype=np.float32)

    # mask classification + general block packing
    maskT = np.ascontiguousarray(mask.T).astype(np.float32)
    cls = classify_mask(maskT, s)
    GEN_INDEX = {}
    gen_blocks = []
    nbk = s // 128
    for i in range(nbk):
        for jj in range(nbk):
            if cls[i, jj] == 2:
                GEN_INDEX[(i, jj)] = len(gen_blocks)
                gen_blocks.append(
                    maskT[128 * i : 128 * (i + 1), 128 * jj : 128 * (jj + 1)]
                )
    n_gen = len(gen_blocks)
    if n_gen:
        maskg = np.ascontiguousarray(np.concatenate(gen_blocks, axis=1))
    else:
        maskg = np.zeros((128, 128), np.float32)

    # softmax shift: exact true max of q k^T / 8 over all heads (BLAS, ~1s)
    qf = x2 @ wq.T
    kf = x2 @ wk.T
    # rope preserves pair norms; compute true scores max per head cheaply
    qh = _rope_np(qf, freqs_cos, freqs_sin)
    kh = _rope_np(kf, freqs_cos, freqs_sin)
    m = 0.0
    for h in range(H):
        qs = qh[:, HD * h : HD * (h + 1)]
        ks = kh[:, HD * h : HD * (h + 1)]
        m = max(m, float(np.abs(qs @ ks.T).max()) / 8.0)
    neg_bias = max(0.0, m - 60.0)

    def pack_w(w_slice):  # [EC, D] -> [128, D] chunked-transpose layout
        t = np.ascontiguousarray(w_slice.T)  # [D, EC=128]
        return np.ascontiguousarray(
            t.reshape(D // 128, 128, 128).transpose(1, 0, 2).reshape(128, D)
        ).astype(bf16)

    ones130 = np.zeros((128, 130), np.float32)
    ones130[:, 0:2] = 1.0
    in_maps = []
    for c in range(NCORES):
        R = slice(EC * c, EC * (c + 1))
        woc = wo[:, R]  # [D, 128]
        woT_pack = np.ascontiguousarray(woc.T).astype(bf16)  # [128, D]
        in_maps.append(
            {
                "xT": xT,
                "cosT": cosT,
                "sinTS": sinTS,
                "wqT": pack_w(wq[R] * 0.125),
                "wkT": pack_w(wk[R]),
                "wvT": pack_w(wv[R]),
                "woT": woT_pack,
                "P128": P.astype(bf16),
                "I128": I.astype(bf16),
                "maskg": maskg,
                "ones2": ones130.astype(bf16),
            }
        )
    return in_maps, cls, n_gen, neg_bias


def _rope_np(t, cos, sin):
    s = t.shape[0]
    tr = t.reshape(s, H, HD // 2, 2)
    te, to = tr[..., 0], tr[..., 1]
    c = cos[:, None, :]
    sn = sin[:, None, :]
    oe = te * c - to * sn
    oo = te * sn + to * c
    return np.stack([oe, oo], axis=-1).reshape(s, H * HD)


def kernel(**inputs):
    from concourse.bass_utils import run_bass_kernel_spmd

    x = np.asarray(inputs["x"], np.float32)
    in_maps, cls, n_gen, neg_bias = host_prep(
        x,
        np.asarray(inputs["freqs_cos"], np.float32),
        np.asarray(inputs["freqs_sin"], np.float32),
        np.asarray(inputs["mask"], np.float32),
        np.asarray(inputs["wq"], np.float32),
        np.asarray(inputs["wk"], np.float32),
        np.asarray(inputs["wv"], np.float32),
        np.asarray(inputs["wo"], np.float32),
    )
    key = (x.shape[1], cls.tobytes(), n_gen, float(neg_bias))
    if key not in _PROGRAM_CACHE:
        _PROGRAM_CACHE[key] = build_program(x.shape[1], cls, n_gen, neg_bias)
    nc = _PROGRAM_CACHE[key]
    res = run_bass_kernel_spmd(nc, in_maps, core_ids=list(range(NCORES)))
    y = np.zeros((x.shape[1], D), np.float32)
    for c in range(NCORES):
        y += np.asarray(res.results[c]["y"], np.float32)
    return y.reshape(x.shape)
